# revision 15
# baseline (speedup 1.0000x reference)
"""Trainium2 Bass kernel for nn_MultiHeadAttention (B=4, S=2048, D=1024, H=16).

Sharding: batch data-parallel x 2-way head tensor-parallel. Core c handles
batch c//2 with heads (c%2)*8..(c%2)*8+7 (512 features). Pairs (2k, 2k+1)
share a batch; after attention each strip's outputs are exchanged with a
2-core AllGather so each core runs the output projection (full 1024-feature
contraction) for its half of the batch sequence (1024 positions).

Per core:
  1. For each 512-query strip s: project K/Q/V for that strip (bf16 matmuls
     over 8 contraction chunks, bias added on the Scalar engine), V is
     PE-transposed into seq-major v_aug with a ones column appended per head
     (softmax denominator rides along the AV matmul).
  2. Causal attention for strip s, 4 head-pairs: scoresT = K-chunk @ Q-group
     into fp32 PSUM (head pairs run concurrently via PE row tiling), exp on
     ACT (or Schraudolph bit-trick exp on DVE for a fraction of groups, to
     split the softmax load across two engines), AV + denominator via one
     accumulating matmul against [V | 1]. Causal boundary chunks are
     triangular-pruned; only the diagonal 128-wide sub-block gets a mask
     multiply.
  3. Per-strip normalization: 8 denominator rows batched into one [8, 512]
     reciprocal, broadcast back to 64 partitions via small selector matmuls,
     multiply, cast bf16, DMA to the exchange staging buffer. One pair
     AllGather (512KB) per strip publishes the strip to the partner core.
  4. Output projection for this core's 1024 sequence positions: indirect
     (index-driven) gather pulls the right strips/ranks out of the gathered
     buffer (indices are per-core inputs so the SPMD program is identical
     across cores), then 8x8 bf16 matmul tiles + bias.
Host wraps: slices weights per core, folds 1/sqrt(dk) into Wq, classifies
mask blocks (drop / keep / causal-staircase / generic), reassembles the
full [4, 2048, 1024] output from the 8 half-batch outputs.
"""

import ml_dtypes
import numpy as np

import concourse.bass as bass
import concourse.bacc as bacc
import concourse.mybir as mybir
import concourse.tile as tile
from concourse.bass_utils import run_bass_kernel_spmd

F32 = mybir.dt.float32
F32R = mybir.dt.float32r
BF16 = mybir.dt.bfloat16
I16 = mybir.dt.int16
I32 = mybir.dt.int32
AF = mybir.ActivationFunctionType
OP = mybir.AluOpType

B, S, D_MODEL, N_HEADS, D_K = 4, 2048, 1024, 16, 64
N_CORES = 8
HL = 8                            # heads per core
FPC = HL * D_K                    # feature slice per core = 512
NHP = HL // 2                     # head pairs per core = 4
S1B = 512                         # query-strip width
S2B = 128                         # key-block height
SP = S // S1B                     # 4 strips
C2 = S // S2B                     # 16 key chunks
KC = D_MODEL // 128               # 8 contraction chunks
GW = 512                          # per-head score-group width (q cols)
TRIW = 128
A_DROP, A_KEEP = -2, -1
A_TRI0 = -3                       # A_TRI0 - j: block valid from col j*128 on

# Schraudolph bf16 exp: bits16 = int16(x * SCH_S + SCH_B); bitcast -> bf16.
# exp fraction handled on DVE (of 8): groups with idx%8 < EXP_DVE8 go to DVE.
EXP_DVE8 = 3
SCH_S = 128.0 / np.log(2.0)
SCH_B = 127.0 * 128.0 - 5.587


def _pack_segs(segs):
    """Bin-pack (i2, kind, qo, w) segments into bins of <= GW q-columns.

    Each bin becomes one score/exp/AV group ([128, 2*GW] PSUM tile)."""
    bins = []
    for sg in sorted(segs, key=lambda s: -s[3]):
        for bn in bins:
            if bn[0] + sg[3] <= GW:
                bn[0] += sg[3]
                bn[1].append(sg)
                break
        else:
            bins.append([sg[3], [sg]])
    return [bn[1] for bn in bins]


_nc_cache = {}


def _build_nc(actions_key, n_masks):
    actions = np.frombuffer(actions_key, dtype=np.int64).reshape(C2, SP)
    nc = bacc.Bacc("TRN2", target_bir_lowering=False, debug=False,
                   num_devices=N_CORES)

    xq = nc.dram_tensor("xq", [128, KC, S], BF16, kind="ExternalInput")
    xk = nc.dram_tensor("xk", [128, KC, S], BF16, kind="ExternalInput")
    xv = nc.dram_tensor("xv", [128, KC, S], BF16, kind="ExternalInput")
    wqkv = nc.dram_tensor("wqkv", [128, KC, 3, FPC], BF16, kind="ExternalInput")
    b3 = nc.dram_tensor("b3", [128, NHP, 3], F32, kind="ExternalInput")
    woT = nc.dram_tensor("woT", [128, KC, KC, 128], BF16, kind="ExternalInput")
    bo = nc.dram_tensor("bo", [128, KC], F32, kind="ExternalInput")
    ident = nc.dram_tensor("ident", [128, 128], BF16, kind="ExternalInput")
    trim = nc.dram_tensor("trim", [TRIW, TRIW], BF16, kind="ExternalInput")
    masks = nc.dram_tensor("masks", [max(n_masks, 1), S2B, S1B], BF16,
                           kind="ExternalInput")
    sel = nc.dram_tensor("sel", [HL, HL, D_K], F32, kind="ExternalInput")
    oidx = nc.dram_tensor("oidx", [2 * KC, 128, 1], I32, kind="ExternalInput")
    agin = nc.dram_tensor("agin", [SP, 128, 4, S1B], BF16)
    agf = nc.dram_tensor("agf", [SP, 2, 128, 4, S1B], BF16)
    out_t = nc.dram_tensor("out_t", [128, KC, 2 * S1B], F32,
                           kind="ExternalOutput")

    rgroups = [[2 * k, 2 * k + 1] for k in range(B)]

    with tile.TileContext(nc) as tc:
      with tc.tile_pool(name="oproj_w", bufs=1) as opw:
        wo_sb = opw.tile([128, KC, KC, 128], BF16, tag="wo")
        bo_sb = opw.tile([128, KC], F32, tag="bo")
        rhs = opw.tile([128, KC, 2 * S1B], BF16, tag="rhs")
        idx_sb = opw.tile([128, 2 * KC], I32, tag="idx")
        with (
            tc.tile_pool(name="const", bufs=1) as cst,
            tc.tile_pool(name="persist", bufs=1) as per,
            tc.tile_pool(name="xin", bufs=5) as xin,
            tc.tile_pool(name="vtmp", bufs=2) as vtmp,
            tc.tile_pool(name="probs", bufs=6) as prp,
            tc.tile_pool(name="avkeep", bufs=12) as avp,
            tc.tile_pool(name="norm", bufs=2) as nrm,
            tc.tile_pool(name="obuf", bufs=4) as obp,
            tc.tile_pool(name="pp_ps", bufs=2, space="PSUM") as pp_ps,
            tc.tile_pool(name="sc_ps", bufs=2, space="PSUM") as sc_ps,
            tc.tile_pool(name="av_ps", bufs=2, space="PSUM") as av_ps,
        ):
            wqkv_sb = cst.tile([128, KC, 3, FPC], BF16, tag="wqkv")
            nc.sync.dma_start(wqkv_sb[:], wqkv[:])
            b3_sb = cst.tile([128, NHP, 3], F32, tag="b3")
            nc.sync.dma_start(b3_sb[:], b3[:])
            id_sb = cst.tile([128, 128], BF16, tag="id")
            nc.sync.dma_start(id_sb[:], ident[:])
            trim_sb = cst.tile([TRIW, TRIW], BF16, tag="trim")
            nc.sync.dma_start(trim_sb[:], trim[:])
            mk_sb = cst.tile([S2B, max(n_masks, 1), S1B], BF16, tag="mk")
            nc.sync.dma_start(mk_sb[:], masks[:].rearrange("n p f -> p n f"))
            sel_f = cst.tile([HL, HL, D_K], F32, tag="self")
            nc.sync.dma_start(sel_f[:], sel[:])
            sel_sb = cst.tile([HL, HL, D_K], F32R, tag="sel")
            nc.vector.tensor_copy(sel_sb[:], sel_f[:])

            qT = per.tile([128, NHP, S], BF16, tag="qT")
            kT = per.tile([128, NHP, S], BF16, tag="kT")
            # V (seq-major), ones column per head for the denominator
            v_aug = per.tile([S2B, C2, HL, D_K + 1], BF16, tag="vaug")
            nc.vector.memset(v_aug[:, :, :, D_K:D_K + 1], 1.0)

            gidx = 0  # global group counter (ACT/DVE exp assignment)
            for s in range(SP):
                scol = slice(s * S1B, (s + 1) * S1B)
                # --- projections for strip s: k, q, v ---
                for t_idx, (name, x_dram) in enumerate(
                        (("k", xk), ("q", xq), ("v", xv))):
                    t_slot = ("q", "k", "v").index(name)
                    xt = xin.tile([128, KC, S1B], BF16, tag="xt")
                    nc.sync.dma_start(xt[:], x_dram[:, :, scol])
                    for ft in range(NHP):
                        ps = pp_ps.tile([128, S1B], F32, tag="pp")
                        for kc in range(KC):
                            nc.tensor.matmul(
                                ps[:], wqkv_sb[:, kc, t_slot,
                                               ft * 128:(ft + 1) * 128],
                                xt[:, kc, :], start=(kc == 0),
                                stop=(kc == KC - 1))
                        b_view = b3_sb[:, ft, t_slot:t_slot + 1]
                        if name == "k":
                            nc.vector.tensor_scalar_add(kT[:, ft, scol],
                                                        ps[:], b_view)
                        elif name == "q":
                            nc.vector.tensor_scalar_add(qT[:, ft, scol],
                                                        ps[:], b_view)
                        else:
                            vt = vtmp.tile([128, S1B], BF16, tag="vt")
                            nc.vector.tensor_scalar_add(vt[:], ps[:], b_view)
                            for j in range(S1B // 128):
                                tp = pp_ps.tile([128, 128], BF16, tag="pp")
                                nc.tensor.transpose(
                                    tp[:], vt[:, j * 128:(j + 1) * 128], id_sb[:])
                                c2 = s * (S1B // 128) + j
                                nc.vector.tensor_copy(
                                    v_aug[:, c2, 2 * ft:2 * ft + 2, 0:D_K],
                                    tp[:].rearrange("p (h d) -> p h d", h=2))

                # --- attention for strip s, 4 head pairs ---
                segs = []
                for i2 in range(C2):
                    a = actions[i2, s]
                    if a == A_DROP:
                        continue
                    if a <= A_TRI0:
                        j = A_TRI0 - a
                        segs.append((i2, "tri", j * TRIW, S1B - j * TRIW))
                    elif a == A_KEEP:
                        segs.append((i2, "keep", 0, S1B))
                    else:
                        segs.append((i2, int(a), 0, S1B))
                groups = _pack_segs(segs)
                nseg = len(segs)
                avcs = [None] * HL
                for hp in range(NHP):
                    avs = []
                    for _lh in range(2):
                        av_t = av_ps.tile([D_K + 1, S1B], F32, tag="av")
                        avs.append(av_t)
                    seg_base = 0
                    pend = None

                    def prv(prt, i16, a, b):
                        ap = prt[:, a:b]
                        return ap.bitcast(BF16) if i16 else ap

                    def emit_av(grp, prt, i16, base, avs=avs, hp=hp):
                        for lh in range(2):
                            off = 0
                            for si, (i2, kind, qo, w) in enumerate(grp):
                                po = lh * GW + off
                                if kind == "tri":
                                    nc.vector.tensor_tensor(
                                        prv(prt, i16, po, po + TRIW),
                                        prv(prt, i16, po, po + TRIW),
                                        trim_sb[:], OP.mult)
                                elif isinstance(kind, int):
                                    nc.vector.tensor_tensor(
                                        prv(prt, i16, po, po + w),
                                        prv(prt, i16, po, po + w),
                                        mk_sb[:, kind, 0:w], OP.mult)
                                nc.tensor.matmul(
                                    avs[lh][:, qo:qo + w],
                                    v_aug[:, i2, 2 * hp + lh, :],
                                    prv(prt, i16, po, po + w),
                                    start=(base + si == 0),
                                    stop=(base + si == nseg - 1))
                                off += w

                    for grp in groups:
                        gw = sum(sg[3] for sg in grp)
                        sc_t = sc_ps.tile([128, 2 * GW], F32, tag="sc")
                        off = 0
                        for (i2, kind, qo, w) in grp:
                            kcol = slice(i2 * S2B, i2 * S2B + S2B)
                            qcol = slice(s * S1B + qo, s * S1B + qo + w)
                            for lh in range(2):
                                r0, r1 = lh * D_K, (lh + 1) * D_K
                                nc.tensor.matmul(
                                    sc_t[:, lh * GW + off:lh * GW + off + w],
                                    kT[r0:r1, hp, kcol], qT[r0:r1, hp, qcol],
                                    start=True, stop=True)
                            off += w
                        use_dve = (gidx % 8) < EXP_DVE8
                        gidx += 1
                        if use_dve:
                            prt = prp.tile([128, 2 * GW], I16, tag="pr")
                            if gw == GW:
                                nc.vector.tensor_scalar(
                                    prt[:], sc_t[:], SCH_S, SCH_B,
                                    OP.mult, OP.add)
                            else:
                                for lh in range(2):
                                    nc.vector.tensor_scalar(
                                        prt[:, lh * GW:lh * GW + gw],
                                        sc_t[:, lh * GW:lh * GW + gw],
                                        SCH_S, SCH_B, OP.mult, OP.add)
                        else:
                            prt = prp.tile([128, 2 * GW], BF16, tag="pr")
                            if gw == GW:
                                nc.scalar.activation(prt[:], sc_t[:], AF.Exp)
                            else:
                                for lh in range(2):
                                    nc.scalar.activation(
                                        prt[:, lh * GW:lh * GW + gw],
                                        sc_t[:, lh * GW:lh * GW + gw], AF.Exp)
                        if pend is not None:
                            emit_av(*pend)
                        pend = (grp, prt, use_dve, seg_base)
                        seg_base += len(grp)
                    if pend is not None:
                        emit_av(*pend)

                    for lh in range(2):
                        avc = avp.tile([D_K + 1, S1B], F32, tag="avc")
                        nc.vector.tensor_copy(avc[:], avs[lh][:])
                        avcs[2 * hp + lh] = avc

                # --- normalization + exchange for strip s ---
                den = nrm.tile([HL, S1B], F32, tag="den")
                for h in range(HL):
                    nc.sync.dma_start(den[h:h + 1, :],
                                      avcs[h][D_K:D_K + 1, :])
                rcp = nrm.tile([HL, S1B], F32, tag="rcp")
                nc.vector.reciprocal_approx_fast(rcp[:], den[:])
                rcpr = nrm.tile([HL, S1B], F32R, tag="rcpr")
                nc.vector.tensor_copy(rcpr[:], rcp[:])
                for h in range(HL):
                    bc = av_ps.tile([D_K, S1B], F32, tag="av")
                    nc.tensor.matmul(bc[:], sel_sb[:, h, :], rcpr[:],
                                     start=True, stop=True)
                    ob = obp.tile([D_K, S1B], BF16, tag="ob")
                    nc.vector.tensor_tensor(ob[:], avcs[h][0:D_K, :], bc[:],
                                            OP.mult)
                    r0 = (h % 2) * D_K
                    nc.sync.dma_start(agin[s, r0:r0 + D_K, h // 2, :], ob[:])
                nc.gpsimd.collective_compute(
                    "AllGather", OP.bypass, ins=[agin[s]], outs=[agf[s]],
                    replica_groups=rgroups)

        # --- output projection for this core's 1024 seq positions ---
        # rhs contraction chunks 0-3 are this core's own heads, gathered from
        # agin (no collective dependency); chunks 4-7 are the partner's,
        # gathered from agf (waits on the pair AllGathers). woT's contraction
        # blocks are permuted per core to match.
        nc.sync.dma_start(wo_sb[:], woT[:])
        nc.sync.dma_start(bo_sb[:], bo[:])
        nc.sync.dma_start(idx_sb[:], oidx[:].rearrange("g p one -> p (g one)"))
        agin_rows = agin[:].rearrange("s p k c -> (s p k) c")
        agf_rows = agf[:].rearrange("s r p k c -> (s r p k) c")
        with (
            tc.tile_pool(name="ob_sb", bufs=3) as ob2p,
            tc.tile_pool(name="op_ps", bufs=8, space="PSUM") as op_ps,
        ):
            for j in range(2):
                for kcg in range(KC // 2):
                    nc.gpsimd.indirect_dma_start(
                        out=rhs[:, kcg, j * S1B:(j + 1) * S1B],
                        out_offset=None, in_=agin_rows,
                        in_offset=bass.IndirectOffsetOnAxis(
                            ap=idx_sb[:, kcg * 2 + j:kcg * 2 + j + 1], axis=0))
            for j in range(2):
                for kcg in range(KC // 2, KC):
                    nc.gpsimd.indirect_dma_start(
                        out=rhs[:, kcg, j * S1B:(j + 1) * S1B],
                        out_offset=None, in_=agf_rows,
                        in_offset=bass.IndirectOffsetOnAxis(
                            ap=idx_sb[:, kcg * 2 + j:kcg * 2 + j + 1], axis=0))
            for j in range(2):
                pss = []
                for dc in range(KC):
                    ps = op_ps.tile([128, S1B], F32, tag="op", name=f"ps{dc}")
                    pss.append(ps)
                for dc in range(KC):
                    for kc in range(KC // 2):
                        nc.tensor.matmul(
                            pss[dc][:], wo_sb[:, kc, dc, :],
                            rhs[:, kc, j * S1B:(j + 1) * S1B],
                            start=(kc == 0), stop=False)
                for dc in range(KC):
                    for kc in range(KC // 2, KC):
                        nc.tensor.matmul(
                            pss[dc][:], wo_sb[:, kc, dc, :],
                            rhs[:, kc, j * S1B:(j + 1) * S1B],
                            start=False, stop=(kc == KC - 1))
                    ob2 = ob2p.tile([128, S1B], F32, tag="obt")
                    nc.vector.tensor_scalar_add(ob2[:], pss[dc][:],
                                                bo_sb[:, dc:dc + 1])
                    nc.sync.dma_start(
                        out_t[:, dc, j * S1B:(j + 1) * S1B], ob2[:])

    nc.finalize()
    return nc


def _classify_mask(mask):
    """Block-classify mask[0,0] on the scoresT grid: per (key-chunk i2,
    query-strip i1) -> drop / keep / index of a unique [128, 512] 0/1 tile."""
    m2 = np.asarray(mask)[0, 0] != 0  # [S, S], m2[q, k]
    actions = np.full((C2, SP), A_DROP, dtype=np.int64)
    uniq, tiles = {}, []
    qs = np.arange(S1B)[None, :]
    ks = np.arange(S2B)[:, None]
    for i2 in range(C2):
        for i1 in range(SP):
            blk = m2[i1 * S1B:(i1 + 1) * S1B, i2 * S2B:(i2 + 1) * S2B].T
            if blk.all():
                actions[i2, i1] = A_KEEP
            elif blk.any():
                j = i2 - (S1B // S2B) * i1
                if 0 <= j < S1B // S2B and np.array_equal(
                        blk, qs >= j * TRIW + ks):
                    actions[i2, i1] = A_TRI0 - j
                    continue
                key = blk.tobytes()
                if key not in uniq:
                    uniq[key] = len(tiles)
                    tiles.append(np.ascontiguousarray(blk).astype(
                        ml_dtypes.bfloat16))
                actions[i2, i1] = uniq[key]
    arr = (np.stack(tiles) if tiles
           else np.zeros((1, S2B, S1B), dtype=ml_dtypes.bfloat16))
    return actions, arr


def _prep(inputs):
    q = np.asarray(inputs["query"], dtype=np.float32)
    k = np.asarray(inputs["key"], dtype=np.float32)
    v = np.asarray(inputs["value"], dtype=np.float32)
    bf = ml_dtypes.bfloat16

    Wq = np.asarray(inputs["Wq"], dtype=np.float32)
    Wk = np.asarray(inputs["Wk"], dtype=np.float32)
    Wv = np.asarray(inputs["Wv"], dtype=np.float32)
    Wo = np.asarray(inputs["Wo"], dtype=np.float32)
    bq = np.asarray(inputs["bq"], dtype=np.float32)
    bk = np.asarray(inputs["bk"], dtype=np.float32)
    bv = np.asarray(inputs["bv"], dtype=np.float32)
    bo = np.asarray(inputs["bo"], dtype=np.float32)

    scale = 1.0 / np.sqrt(D_K)
    actions, mask_tiles = _classify_mask(inputs["mask"])

    # exp-overflow guard for the no-max-subtract softmax (Cauchy-Schwarz)
    q2 = q.reshape(B * S, D_MODEL)
    k2 = k.reshape(B * S, D_MODEL)
    qn = q2 @ Wq.T + bq
    kn = k2 @ Wk.T + bk
    qmax = np.linalg.norm(qn.reshape(-1, N_HEADS, D_K), axis=-1).max()
    kmax = np.linalg.norm(kn.reshape(-1, N_HEADS, D_K), axis=-1).max()
    assert scale * qmax * kmax < 80.0, "score bound too large for exp"

    WoT = np.ascontiguousarray(Wo.T)  # [feat, dout]
    shared = {
        "bo": np.ascontiguousarray(bo.reshape(KC, 128).T),
        "ident": np.eye(128, dtype=np.float32).astype(bf),
        "trim": np.ascontiguousarray(
            (np.arange(TRIW)[None, :] >= np.arange(TRIW)[:, None])
            .astype(np.float32)).astype(bf),
        "masks": mask_tiles,
        "sel": np.ascontiguousarray(
            (np.eye(HL, dtype=np.float32)[:, :, None]
             * np.ones((1, 1, D_K), dtype=np.float32))),
    }
    in_maps = []
    pp = np.arange(128)
    for c in range(N_CORES):
        b, hh = c // 2, c % 2
        sl = slice(hh * FPC, (hh + 1) * FPC)
        m = dict(shared)
        for nm, arr in (("xq", q[b]), ("xk", k[b]), ("xv", v[b])):
            # [S, D] -> feature-chunk-major [128, KC, S]
            m[nm] = np.ascontiguousarray(
                arr.T.reshape(KC, 128, S).transpose(1, 0, 2)).astype(bf)
        wq3 = np.concatenate(
            [(Wq[sl] * scale).T, Wk[sl].T, Wv[sl].T], axis=1)  # [D, 3F]
        m["wqkv"] = np.ascontiguousarray(
            wq3.reshape(KC, 128, 3, FPC).transpose(1, 0, 2, 3)).astype(bf)
        m["b3"] = np.ascontiguousarray(np.stack(
            [bq[sl] * scale, bk[sl], bv[sl]], axis=1).astype(np.float32)
            .reshape(NHP, 128, 3).transpose(1, 0, 2))
        # woT contraction chunks permuted per core: 0-3 = own feature chunks,
        # 4-7 = partner's.
        perm = [hh * 4 + i for i in range(4)] + [(1 - hh) * 4 + i
                                                for i in range(4)]
        WoTp = WoT.reshape(KC, 128, D_MODEL)[perm].reshape(D_MODEL, D_MODEL)
        m["woT"] = np.ascontiguousarray(
            WoTp.reshape(KC, 128, KC, 128).transpose(1, 0, 2, 3)).astype(bf)
        # rhs gather indices. Local chunks (0-3) read agin rows
        # (s*128 + p)*4 + kcl; partner chunks (4-7) read agf rows
        # ((s*2 + r)*128 + p)*4 + kcl.
        idx = np.empty((2 * KC, 128), dtype=np.int32)
        for kcg in range(KC):
            for j in range(2):
                st = 2 * hh + j
                if kcg < 4:
                    idx[kcg * 2 + j] = (st * 128 + pp) * 4 + kcg
                else:
                    idx[kcg * 2 + j] = ((st * 2 + (1 - hh)) * 128 + pp) * 4 \
                        + (kcg - 4)
        m["oidx"] = np.ascontiguousarray(idx.reshape(2 * KC, 128, 1))
        in_maps.append(m)
    return in_maps, actions, mask_tiles


def _run(inputs, trace=False, trace_cores=None):
    in_maps, actions, mask_tiles = _prep(inputs)
    key = (actions.tobytes(), len(mask_tiles))
    if key not in _nc_cache:
        _nc_cache[key] = _build_nc(key[0], key[1])
    nc = _nc_cache[key]
    res = run_bass_kernel_spmd(nc, in_maps, list(range(N_CORES)),
                               trace=trace, trace_cores=trace_cores)
    out = np.empty((B, S, D_MODEL), dtype=np.float32)
    for c in range(N_CORES):
        b, hh = c // 2, c % 2
        o = res.results[c]["out_t"]  # [128, KC, 1024]
        blk = o.transpose(2, 1, 0).reshape(2 * S1B, D_MODEL)
        out[b, hh * 2 * S1B:(hh + 1) * 2 * S1B, :] = blk
    return out, res


def kernel(**inputs) -> np.ndarray:
    out, _ = _run(inputs)
    return out


# revision 24
# speedup vs baseline: 1.0594x; 1.0594x over previous
"""Trainium2 Bass kernel for nn_MultiHeadAttention (B=4, S=2048, D=1024, H=16).

Sharding: batch data-parallel x 2-way head tensor-parallel. Core c handles
batch c//2 with heads (c%2)*8..(c%2)*8+7 (512 features). Pairs (2k, 2k+1)
share a batch; after attention each strip's outputs are exchanged with a
2-core AllGather so each core runs the output projection (full 1024-feature
contraction) for its half of the batch sequence (1024 positions).

Per core:
  1. For each 512-query strip s: project K/Q/V for that strip (bf16 matmuls
     over 8 contraction chunks, bias added on the Scalar engine), V is
     PE-transposed into seq-major v_aug with a ones column appended per head
     (softmax denominator rides along the AV matmul).
  2. Causal attention for strip s, 4 head-pairs: scoresT = K-chunk @ Q-group
     into fp32 PSUM (head pairs run concurrently via PE row tiling), exp on
     ACT (or Schraudolph bit-trick exp on DVE for a fraction of groups, to
     split the softmax load across two engines), AV + denominator via one
     accumulating matmul against [V | 1]. Causal boundary chunks are
     triangular-pruned; only the diagonal 128-wide sub-block gets a mask
     multiply.
  3. Per-strip normalization: 8 denominator rows batched into one [8, 512]
     reciprocal, broadcast back to 64 partitions via small selector matmuls,
     multiply, cast bf16, DMA to the exchange staging buffer. One pair
     AllGather (512KB) per strip publishes the strip to the partner core.
  4. Output projection for this core's 1024 sequence positions: indirect
     (index-driven) gather pulls the right strips/ranks out of the gathered
     buffer (indices are per-core inputs so the SPMD program is identical
     across cores), then 8x8 bf16 matmul tiles + bias.
Host wraps: slices weights per core, folds 1/sqrt(dk) into Wq, classifies
mask blocks (drop / keep / causal-staircase / generic), reassembles the
full [4, 2048, 1024] output from the 8 half-batch outputs.
"""

import ml_dtypes
import numpy as np

import concourse.bass as bass
import concourse.bacc as bacc
import concourse.mybir as mybir
import concourse.tile as tile
from concourse.bass_utils import run_bass_kernel_spmd

F32 = mybir.dt.float32
F32R = mybir.dt.float32r
BF16 = mybir.dt.bfloat16
I16 = mybir.dt.int16
I32 = mybir.dt.int32
AF = mybir.ActivationFunctionType
OP = mybir.AluOpType

B, S, D_MODEL, N_HEADS, D_K = 4, 2048, 1024, 16, 64
N_CORES = 8
HL = 8                            # heads per core
FPC = HL * D_K                    # feature slice per core = 512
NHP = HL // 2                     # head pairs per core = 4
S1B = 512                         # query-strip width
S2B = 128                         # key-block height
SP = S // S1B                     # 4 strips
C2 = S // S2B                     # 16 key chunks
KC = D_MODEL // 128               # 8 contraction chunks
GW = 512                          # per-head score-group width (q cols)
TRIW = 128
A_DROP, A_KEEP = -2, -1
A_TRI0 = -3                       # A_TRI0 - j: block valid from col j*128 on

# Schraudolph bf16 exp: bits16 = int16(x * SCH_S + SCH_B); bitcast -> bf16.
# exp fraction handled on DVE (of 8): groups with idx%8 < EXP_DVE8 go to DVE.
EXP_DVE8 = 0
SCH_S = 128.0 / np.log(2.0)
SCH_B = 127.0 * 128.0 - 5.587


def _pack_segs(segs):
    """Bin-pack (i2, kind, qo, w) segments into bins of <= GW q-columns.

    Each bin becomes one score/exp/AV group ([128, 2*GW] PSUM tile)."""
    bins = []
    for sg in sorted(segs, key=lambda s: -s[3]):
        for bn in bins:
            if bn[0] + sg[3] <= GW:
                bn[0] += sg[3]
                bn[1].append(sg)
                break
        else:
            bins.append([sg[3], [sg]])
    return [bn[1] for bn in bins]


_nc_cache = {}


def _build_nc(actions_key, n_masks):
    actions = np.frombuffer(actions_key, dtype=np.int64).reshape(C2, SP)
    nc = bacc.Bacc("TRN2", target_bir_lowering=False, debug=False,
                   num_devices=N_CORES)

    xq = nc.dram_tensor("xq", [128, KC, S], BF16, kind="ExternalInput")
    xk = nc.dram_tensor("xk", [128, KC, S], BF16, kind="ExternalInput")
    xv = nc.dram_tensor("xv", [128, KC, S], BF16, kind="ExternalInput")
    wqkv = nc.dram_tensor("wqkv", [128, KC, 3, FPC], BF16, kind="ExternalInput")
    b3 = nc.dram_tensor("b3", [128, NHP, 3], F32, kind="ExternalInput")
    woT = nc.dram_tensor("woT", [128, KC, KC, 128], BF16, kind="ExternalInput")
    bo = nc.dram_tensor("bo", [128, KC], F32, kind="ExternalInput")
    ident = nc.dram_tensor("ident", [128, 128], BF16, kind="ExternalInput")
    trim = nc.dram_tensor("trim", [TRIW, TRIW], BF16, kind="ExternalInput")
    masks = nc.dram_tensor("masks", [max(n_masks, 1), S2B, S1B], BF16,
                           kind="ExternalInput")
    sel = nc.dram_tensor("sel", [HL, HL, D_K], F32, kind="ExternalInput")
    sel2 = nc.dram_tensor("sel2", [2, 2, D_K], F32, kind="ExternalInput")
    oidx = nc.dram_tensor("oidx", [2 * KC, 128, 1], I32, kind="ExternalInput")
    # agin rows: (s, kcl, p); agfX rows: strips 0-2 whole-strip AllGathers at
    # block s*8 + r*4 + kcl, strip 3 per-head-pair AllGathers at block
    # 24 + hp*2 + r (so the last collective fires right after hp3 finishes).
    agin = nc.dram_tensor("agin", [SP, 4, 128, S1B], BF16)
    agfX = nc.dram_tensor("agfX", [32, 128, S1B], BF16)
    out_t = nc.dram_tensor("out_t", [128, KC, 2 * S1B], F32,
                           kind="ExternalOutput")

    rgroups = [[2 * k, 2 * k + 1] for k in range(B)]

    with tile.TileContext(nc) as tc:
      with tc.tile_pool(name="oproj_w", bufs=1) as opw:
        wo_sb = opw.tile([128, KC, KC, 128], BF16, tag="wo")
        bo_sb = opw.tile([128, KC], F32, tag="bo")
        rhs = opw.tile([128, KC, 2 * S1B], BF16, tag="rhs")
        idx_sb = opw.tile([128, 2 * KC], I32, tag="idx")
        with (
            tc.tile_pool(name="const", bufs=1) as cst,
            tc.tile_pool(name="persist", bufs=1) as per,
            tc.tile_pool(name="xin", bufs=4) as xin,
            tc.tile_pool(name="vtmp", bufs=2) as vtmp,
            tc.tile_pool(name="probs", bufs=6) as prp,
            tc.tile_pool(name="avkeep", bufs=12) as avp,
            tc.tile_pool(name="norm", bufs=2) as nrm,
            tc.tile_pool(name="obuf", bufs=4) as obp,
            tc.tile_pool(name="pp_ps", bufs=2, space="PSUM") as pp_ps,
            tc.tile_pool(name="sc_ps", bufs=2, space="PSUM") as sc_ps,
            tc.tile_pool(name="av_ps", bufs=2, space="PSUM") as av_ps,
        ):
            wqkv_sb = cst.tile([128, KC, 3, FPC], BF16, tag="wqkv")
            nc.sync.dma_start(wqkv_sb[:], wqkv[:])
            b3_sb = cst.tile([128, NHP, 3], F32, tag="b3")
            nc.sync.dma_start(b3_sb[:], b3[:])
            id_sb = cst.tile([128, 128], BF16, tag="id")
            nc.sync.dma_start(id_sb[:], ident[:])
            trim_sb = cst.tile([TRIW, TRIW], BF16, tag="trim")
            nc.sync.dma_start(trim_sb[:], trim[:])
            mk_sb = cst.tile([S2B, max(n_masks, 1), S1B], BF16, tag="mk")
            nc.sync.dma_start(mk_sb[:], masks[:].rearrange("n p f -> p n f"))
            sel_f = cst.tile([HL, HL, D_K], F32, tag="self")
            nc.sync.dma_start(sel_f[:], sel[:])
            sel_sb = cst.tile([HL, HL, D_K], F32R, tag="sel")
            nc.vector.tensor_copy(sel_sb[:], sel_f[:])
            sel2_f = cst.tile([2, 2, D_K], F32, tag="sel2f")
            nc.sync.dma_start(sel2_f[:], sel2[:])
            sel2_sb = cst.tile([2, 2, D_K], F32R, tag="sel2")
            nc.vector.tensor_copy(sel2_sb[:], sel2_f[:])

            qT = per.tile([128, NHP, S], BF16, tag="qT")
            kT = per.tile([128, NHP, S], BF16, tag="kT")
            # V (seq-major), ones column per head for the denominator
            v_aug = per.tile([S2B, C2, HL, D_K + 1], BF16, tag="vaug")
            nc.vector.memset(v_aug[:, :, :, D_K:D_K + 1], 1.0)

            gidx = 0  # global group counter (ACT/DVE exp assignment)
            for s in range(SP):
                scol = slice(s * S1B, (s + 1) * S1B)
                # --- projections for strip s: k, q, v ---
                for t_idx, (name, x_dram) in enumerate(
                        (("k", xk), ("q", xq), ("v", xv))):
                    t_slot = ("q", "k", "v").index(name)
                    xt = xin.tile([128, KC, S1B], BF16, tag="xt")
                    nc.sync.dma_start(xt[:], x_dram[:, :, scol])
                    for ft in range(NHP):
                        ps = pp_ps.tile([128, S1B], F32, tag="pp")
                        for kc in range(KC):
                            nc.tensor.matmul(
                                ps[:], wqkv_sb[:, kc, t_slot,
                                               ft * 128:(ft + 1) * 128],
                                xt[:, kc, :], start=(kc == 0),
                                stop=(kc == KC - 1))
                        b_view = b3_sb[:, ft, t_slot:t_slot + 1]
                        if name == "k":
                            nc.vector.tensor_scalar_add(kT[:, ft, scol],
                                                        ps[:], b_view)
                        elif name == "q":
                            nc.vector.tensor_scalar_add(qT[:, ft, scol],
                                                        ps[:], b_view)
                        else:
                            vt = vtmp.tile([128, S1B], BF16, tag="vt")
                            nc.vector.tensor_scalar_add(vt[:], ps[:], b_view)
                            for j in range(S1B // 128):
                                tp = pp_ps.tile([128, 128], BF16, tag="pp")
                                nc.tensor.transpose(
                                    tp[:], vt[:, j * 128:(j + 1) * 128], id_sb[:])
                                c2 = s * (S1B // 128) + j
                                nc.vector.tensor_copy(
                                    v_aug[:, c2, 2 * ft:2 * ft + 2, 0:D_K],
                                    tp[:].rearrange("p (h d) -> p h d", h=2))

                # --- attention for strip s, 4 head pairs ---
                segs = []
                for i2 in range(C2):
                    a = actions[i2, s]
                    if a == A_DROP:
                        continue
                    if a <= A_TRI0:
                        j = A_TRI0 - a
                        segs.append((i2, "tri", j * TRIW, S1B - j * TRIW))
                    elif a == A_KEEP:
                        segs.append((i2, "keep", 0, S1B))
                    else:
                        segs.append((i2, int(a), 0, S1B))
                groups = _pack_segs(segs)
                nseg = len(segs)
                avcs = [None] * HL
                for hp in range(NHP):
                    avs = []
                    for _lh in range(2):
                        av_t = av_ps.tile([D_K + 1, S1B], F32, tag="av")
                        avs.append(av_t)
                    seg_base = 0
                    pend = None

                    def prv(prt, i16, a, b):
                        ap = prt[:, a:b]
                        return ap.bitcast(BF16) if i16 else ap

                    def emit_av(grp, prt, i16, base, avs=avs, hp=hp):
                        for lh in range(2):
                            off = 0
                            for si, (i2, kind, qo, w) in enumerate(grp):
                                po = lh * GW + off
                                if kind == "tri":
                                    nc.vector.tensor_tensor(
                                        prv(prt, i16, po, po + TRIW),
                                        prv(prt, i16, po, po + TRIW),
                                        trim_sb[:], OP.mult)
                                elif isinstance(kind, int):
                                    nc.vector.tensor_tensor(
                                        prv(prt, i16, po, po + w),
                                        prv(prt, i16, po, po + w),
                                        mk_sb[:, kind, 0:w], OP.mult)
                                nc.tensor.matmul(
                                    avs[lh][:, qo:qo + w],
                                    v_aug[:, i2, 2 * hp + lh, :],
                                    prv(prt, i16, po, po + w),
                                    start=(base + si == 0),
                                    stop=(base + si == nseg - 1))
                                off += w

                    for grp in groups:
                        gw = sum(sg[3] for sg in grp)
                        sc_t = sc_ps.tile([128, 2 * GW], F32, tag="sc")
                        off = 0
                        for (i2, kind, qo, w) in grp:
                            kcol = slice(i2 * S2B, i2 * S2B + S2B)
                            qcol = slice(s * S1B + qo, s * S1B + qo + w)
                            for lh in range(2):
                                r0, r1 = lh * D_K, (lh + 1) * D_K
                                nc.tensor.matmul(
                                    sc_t[:, lh * GW + off:lh * GW + off + w],
                                    kT[r0:r1, hp, kcol], qT[r0:r1, hp, qcol],
                                    start=True, stop=True)
                            off += w
                        use_dve = (gidx % 8) < EXP_DVE8
                        gidx += 1
                        if use_dve:
                            prt = prp.tile([128, 2 * GW], I16, tag="pr")
                            if gw == GW:
                                nc.vector.tensor_scalar(
                                    prt[:], sc_t[:], SCH_S, SCH_B,
                                    OP.mult, OP.add)
                            else:
                                for lh in range(2):
                                    nc.vector.tensor_scalar(
                                        prt[:, lh * GW:lh * GW + gw],
                                        sc_t[:, lh * GW:lh * GW + gw],
                                        SCH_S, SCH_B, OP.mult, OP.add)
                        else:
                            prt = prp.tile([128, 2 * GW], BF16, tag="pr")
                            if gw == GW:
                                nc.scalar.activation(prt[:], sc_t[:], AF.Exp)
                            else:
                                for lh in range(2):
                                    nc.scalar.activation(
                                        prt[:, lh * GW:lh * GW + gw],
                                        sc_t[:, lh * GW:lh * GW + gw], AF.Exp)
                        if pend is not None:
                            emit_av(*pend)
                        pend = (grp, prt, use_dve, seg_base)
                        seg_base += len(grp)
                    if pend is not None:
                        emit_av(*pend)

                    for lh in range(2):
                        avc = avp.tile([D_K + 1, S1B], F32, tag="avc")
                        nc.vector.tensor_copy(avc[:], avs[lh][:])
                        avcs[2 * hp + lh] = avc

                    if s == SP - 1:
                        # last strip: normalize + exchange per head pair so
                        # the final (tail-gating) collective is small and
                        # fires as soon as hp finishes
                        den = nrm.tile([2, S1B], F32, tag="den")
                        for lh in range(2):
                            nc.sync.dma_start(
                                den[lh:lh + 1, :],
                                avcs[2 * hp + lh][D_K:D_K + 1, :])
                        rcp = nrm.tile([2, S1B], F32, tag="rcp")
                        nc.vector.reciprocal_approx_fast(rcp[:], den[:])
                        rcpr = nrm.tile([2, S1B], F32R, tag="rcpr")
                        nc.vector.tensor_copy(rcpr[:], rcp[:])
                        for lh in range(2):
                            bc = av_ps.tile([D_K, S1B], F32, tag="av")
                            nc.tensor.matmul(bc[:], sel2_sb[:, lh, :],
                                             rcpr[:], start=True, stop=True)
                            ob = obp.tile([D_K, S1B], BF16, tag="ob")
                            nc.vector.tensor_tensor(
                                ob[:], avcs[2 * hp + lh][0:D_K, :], bc[:],
                                OP.mult)
                            nc.sync.dma_start(
                                agin[s, hp, lh * D_K:(lh + 1) * D_K, :],
                                ob[:])
                        nc.gpsimd.collective_compute(
                            "AllGather", OP.bypass, ins=[agin[s, hp]],
                            outs=[agfX[24 + 2 * hp:26 + 2 * hp]],
                            replica_groups=rgroups)

                if s < SP - 1:
                    # --- batched normalization + one exchange for strip s ---
                    den = nrm.tile([HL, S1B], F32, tag="den")
                    for h in range(HL):
                        nc.sync.dma_start(den[h:h + 1, :],
                                          avcs[h][D_K:D_K + 1, :])
                    rcp = nrm.tile([HL, S1B], F32, tag="rcp")
                    nc.vector.reciprocal_approx_fast(rcp[:], den[:])
                    rcpr = nrm.tile([HL, S1B], F32R, tag="rcpr")
                    nc.vector.tensor_copy(rcpr[:], rcp[:])
                    for h in range(HL):
                        bc = av_ps.tile([D_K, S1B], F32, tag="av")
                        nc.tensor.matmul(bc[:], sel_sb[:, h, :], rcpr[:],
                                         start=True, stop=True)
                        ob = obp.tile([D_K, S1B], BF16, tag="ob")
                        nc.vector.tensor_tensor(ob[:], avcs[h][0:D_K, :],
                                                bc[:], OP.mult)
                        r0 = (h % 2) * D_K
                        nc.sync.dma_start(
                            agin[s, h // 2, r0:r0 + D_K, :], ob[:])
                    nc.gpsimd.collective_compute(
                        "AllGather", OP.bypass, ins=[agin[s]],
                        outs=[agfX[s * 8:(s + 1) * 8]],
                        replica_groups=rgroups)

        # --- output projection for this core's 1024 seq positions ---
        # rhs contraction chunks 0-3 are this core's own heads, gathered from
        # agin (no collective dependency); chunks 4-7 are the partner's,
        # gathered from agf (waits on the pair AllGathers). woT's contraction
        # blocks are permuted per core to match.
        nc.sync.dma_start(wo_sb[:], woT[:])
        nc.sync.dma_start(bo_sb[:], bo[:])
        nc.sync.dma_start(idx_sb[:], oidx[:].rearrange("g p one -> p (g one)"))
        agin_rows = agin[:].rearrange("s k p c -> (s k p) c")
        agf_rows = agfX[:].rearrange("w p c -> (w p) c")
        with (
            tc.tile_pool(name="ob_sb", bufs=3) as ob2p,
            tc.tile_pool(name="op_ps", bufs=8, space="PSUM") as op_ps,
        ):
            for j in range(2):
                for kcg in range(KC // 2):
                    nc.gpsimd.indirect_dma_start(
                        out=rhs[:, kcg, j * S1B:(j + 1) * S1B],
                        out_offset=None, in_=agin_rows,
                        in_offset=bass.IndirectOffsetOnAxis(
                            ap=idx_sb[:, kcg * 2 + j:kcg * 2 + j + 1], axis=0))
            for j in range(2):
                for kcg in range(KC // 2, KC):
                    nc.gpsimd.indirect_dma_start(
                        out=rhs[:, kcg, j * S1B:(j + 1) * S1B],
                        out_offset=None, in_=agf_rows,
                        in_offset=bass.IndirectOffsetOnAxis(
                            ap=idx_sb[:, kcg * 2 + j:kcg * 2 + j + 1], axis=0))
            for j in range(2):
                pss = []
                for dc in range(KC):
                    ps = op_ps.tile([128, S1B], F32, tag="op", name=f"ps{dc}")
                    pss.append(ps)
                for dc in range(KC):
                    for kc in range(KC // 2):
                        nc.tensor.matmul(
                            pss[dc][:], wo_sb[:, kc, dc, :],
                            rhs[:, kc, j * S1B:(j + 1) * S1B],
                            start=(kc == 0), stop=False)
                for dc in range(KC):
                    for kc in range(KC // 2, KC):
                        nc.tensor.matmul(
                            pss[dc][:], wo_sb[:, kc, dc, :],
                            rhs[:, kc, j * S1B:(j + 1) * S1B],
                            start=False, stop=(kc == KC - 1))
                    ob2 = ob2p.tile([128, S1B], F32, tag="obt")
                    nc.vector.tensor_scalar_add(ob2[:], pss[dc][:],
                                                bo_sb[:, dc:dc + 1])
                    nc.sync.dma_start(
                        out_t[:, dc, j * S1B:(j + 1) * S1B], ob2[:])

    nc.finalize()
    return nc


def _classify_mask(mask):
    """Block-classify mask[0,0] on the scoresT grid: per (key-chunk i2,
    query-strip i1) -> drop / keep / index of a unique [128, 512] 0/1 tile."""
    m2 = np.asarray(mask)[0, 0] != 0  # [S, S], m2[q, k]
    actions = np.full((C2, SP), A_DROP, dtype=np.int64)
    uniq, tiles = {}, []
    qs = np.arange(S1B)[None, :]
    ks = np.arange(S2B)[:, None]
    for i2 in range(C2):
        for i1 in range(SP):
            blk = m2[i1 * S1B:(i1 + 1) * S1B, i2 * S2B:(i2 + 1) * S2B].T
            if blk.all():
                actions[i2, i1] = A_KEEP
            elif blk.any():
                j = i2 - (S1B // S2B) * i1
                if 0 <= j < S1B // S2B and np.array_equal(
                        blk, qs >= j * TRIW + ks):
                    actions[i2, i1] = A_TRI0 - j
                    continue
                key = blk.tobytes()
                if key not in uniq:
                    uniq[key] = len(tiles)
                    tiles.append(np.ascontiguousarray(blk).astype(
                        ml_dtypes.bfloat16))
                actions[i2, i1] = uniq[key]
    arr = (np.stack(tiles) if tiles
           else np.zeros((1, S2B, S1B), dtype=ml_dtypes.bfloat16))
    return actions, arr


def _prep(inputs):
    q = np.asarray(inputs["query"], dtype=np.float32)
    k = np.asarray(inputs["key"], dtype=np.float32)
    v = np.asarray(inputs["value"], dtype=np.float32)
    bf = ml_dtypes.bfloat16

    Wq = np.asarray(inputs["Wq"], dtype=np.float32)
    Wk = np.asarray(inputs["Wk"], dtype=np.float32)
    Wv = np.asarray(inputs["Wv"], dtype=np.float32)
    Wo = np.asarray(inputs["Wo"], dtype=np.float32)
    bq = np.asarray(inputs["bq"], dtype=np.float32)
    bk = np.asarray(inputs["bk"], dtype=np.float32)
    bv = np.asarray(inputs["bv"], dtype=np.float32)
    bo = np.asarray(inputs["bo"], dtype=np.float32)

    scale = 1.0 / np.sqrt(D_K)
    actions, mask_tiles = _classify_mask(inputs["mask"])

    # exp-overflow guard for the no-max-subtract softmax (Cauchy-Schwarz)
    q2 = q.reshape(B * S, D_MODEL)
    k2 = k.reshape(B * S, D_MODEL)
    qn = q2 @ Wq.T + bq
    kn = k2 @ Wk.T + bk
    qmax = np.linalg.norm(qn.reshape(-1, N_HEADS, D_K), axis=-1).max()
    kmax = np.linalg.norm(kn.reshape(-1, N_HEADS, D_K), axis=-1).max()
    assert scale * qmax * kmax < 80.0, "score bound too large for exp"

    WoT = np.ascontiguousarray(Wo.T)  # [feat, dout]
    shared = {
        "bo": np.ascontiguousarray(bo.reshape(KC, 128).T),
        "ident": np.eye(128, dtype=np.float32).astype(bf),
        "trim": np.ascontiguousarray(
            (np.arange(TRIW)[None, :] >= np.arange(TRIW)[:, None])
            .astype(np.float32)).astype(bf),
        "masks": mask_tiles,
        "sel": np.ascontiguousarray(
            (np.eye(HL, dtype=np.float32)[:, :, None]
             * np.ones((1, 1, D_K), dtype=np.float32))),
        "sel2": np.ascontiguousarray(
            (np.eye(2, dtype=np.float32)[:, :, None]
             * np.ones((1, 1, D_K), dtype=np.float32))),
    }
    in_maps = []
    pp = np.arange(128)
    for c in range(N_CORES):
        b, hh = c // 2, c % 2
        sl = slice(hh * FPC, (hh + 1) * FPC)
        m = dict(shared)
        for nm, arr in (("xq", q[b]), ("xk", k[b]), ("xv", v[b])):
            # [S, D] -> feature-chunk-major [128, KC, S]
            m[nm] = np.ascontiguousarray(
                arr.T.reshape(KC, 128, S).transpose(1, 0, 2)).astype(bf)
        wq3 = np.concatenate(
            [(Wq[sl] * scale).T, Wk[sl].T, Wv[sl].T], axis=1)  # [D, 3F]
        m["wqkv"] = np.ascontiguousarray(
            wq3.reshape(KC, 128, 3, FPC).transpose(1, 0, 2, 3)).astype(bf)
        m["b3"] = np.ascontiguousarray(np.stack(
            [bq[sl] * scale, bk[sl], bv[sl]], axis=1).astype(np.float32)
            .reshape(NHP, 128, 3).transpose(1, 0, 2))
        # woT contraction chunks permuted per core: 0-3 = own feature chunks,
        # 4-7 = partner's.
        perm = [hh * 4 + i for i in range(4)] + [(1 - hh) * 4 + i
                                                for i in range(4)]
        WoTp = WoT.reshape(KC, 128, D_MODEL)[perm].reshape(D_MODEL, D_MODEL)
        m["woT"] = np.ascontiguousarray(
            WoTp.reshape(KC, 128, KC, 128).transpose(1, 0, 2, 3)).astype(bf)
        # rhs gather indices. Local chunks (0-3) read agin rows
        # (s*4 + kcl)*128 + p; partner chunks (4-7) read agfX rows w*128 + p
        # with w = s*8 + r*4 + kcl for strips 0-2, w = 24 + kcl*2 + r for
        # strip 3 (per-head-pair collectives).
        rp = 1 - hh
        idx = np.empty((2 * KC, 128), dtype=np.int32)
        for kcg in range(KC):
            for j in range(2):
                st = 2 * hh + j
                if kcg < 4:
                    idx[kcg * 2 + j] = (st * 4 + kcg) * 128 + pp
                else:
                    kcl = kcg - 4
                    if st < SP - 1:
                        w = st * 8 + rp * 4 + kcl
                    else:
                        w = 24 + kcl * 2 + rp
                    idx[kcg * 2 + j] = w * 128 + pp
        m["oidx"] = np.ascontiguousarray(idx.reshape(2 * KC, 128, 1))
        in_maps.append(m)
    return in_maps, actions, mask_tiles


def _run(inputs, trace=False, trace_cores=None):
    in_maps, actions, mask_tiles = _prep(inputs)
    key = (actions.tobytes(), len(mask_tiles))
    if key not in _nc_cache:
        _nc_cache[key] = _build_nc(key[0], key[1])
    nc = _nc_cache[key]
    res = run_bass_kernel_spmd(nc, in_maps, list(range(N_CORES)),
                               trace=trace, trace_cores=trace_cores)
    out = np.empty((B, S, D_MODEL), dtype=np.float32)
    for c in range(N_CORES):
        b, hh = c // 2, c % 2
        o = res.results[c]["out_t"]  # [128, KC, 1024]
        blk = o.transpose(2, 1, 0).reshape(2 * S1B, D_MODEL)
        out[b, hh * 2 * S1B:(hh + 1) * 2 * S1B, :] = blk
    return out, res


def kernel(**inputs) -> np.ndarray:
    out, _ = _run(inputs)
    return out
